# revision 50
# baseline (speedup 1.0000x reference)
"""Trainium2 Bass kernel for nn_CrossAttention3D (B=4, C=D=512, H=W=64).

Strategy
--------
reference:  x=(b,c,s) with s=h*w=4096;  Q/K/V = per-pixel linear (1x1 conv),
            sim = Q K^T * D^-0.5, attn = softmax(sim), o = attn V,
            y = o Wo^T + bo.

Sharding: 8 cores = (batch b in 0..3) x (query-half in 0..1); each core does
attention + output for its 2048 query tokens. No collectives.

Algebraic refactor (host folds weight-weight products, exact math):
  sim[t,s] = x_t^T H xq_s + x_t^T wt + c_s;  H = Wk^T (Wq*scale),
  wt = Wk^T (bq*scale); the c_s term cancels in softmax.
  U = H xq + wt;  P = exp(x^T U);  Z = sum_t x_t P[t,s];  l = ones^T P
  y = W2 (Z/l) + bo'  with  W2 = Wo Wv, bo' = Wo bv + bo.

All matmuls are fp8e4 DoubleRow (256-deep per 512-cycle pass = 2x bf16).
Scales: ht*512, U*64 (undone in the exp scale), w2t*32, l-ones = 0.125 so
rlb = 1/pl = 8/l and z = Z*rlb = 8*(Z/l) sits in e4m3 range;
y = py/(8*32) + bo'. Emulated rel err ~9.4e-3 vs the 2e-2 gate.

Key scheduling facts (measured):
  - ~7.2us framework preamble before any user instruction.
  - PE clock ramps to full over ~5.5us of CONTINUOUS execution; any idle
    gap resets it. Warm-up matmuls bridge the DMA head.
  - DMA triggers cost ~650ns each, serialized per engine; sync AND scalar
    are both HWDGE trigger engines, used in parallel.
  - A DMA queue gets ~1/nactive of ~330GB/s, so the head ships in 4 small
    descriptors before the bulk stream starts.

Layout: the host permutes x tiles per core so the 4 QUERY tiles come
first; U's moving data IS x tiles 0-3 (no separate xq tensor). x tile 0 +
ht ship inside `head`, split by contraction half so U(0)'s j=0 passes
start after half the head. U(0) runs j-outer with 4 PSUM accumulators
(3 from ps + the idle psl bank); U(st>=1) is emitted just-in-time before
st's sim units (30us-late deadlines). U evacs alternate ACT/DVE.

l is accumulated as DoubleRow matmuls with BROADCAST ones weights
[P,2,128] -> pl is [128,512] at identical PE cost (a pass is bound by its
512-wide moving stream), so the DVE reciprocal needs no partition
broadcast. The exp tiles are tree-summed in f8 on the mostly-idle gpsimd
(+DVE for mid-st adds) before the l matmul touches them: 4-pair groups,
plus an 8-pair root for each st's first half, so the PE runs ~14 l
passes instead of 64 (boundary-adjacent adds stay on gpsimd — a DVE add
there queues ahead of the next U evacs and stalls the next tile's sim).
rl premultiplies into the Z evacuation (z = po*rlb). On the LAST tile
the l matmuls go first (recip overlaps the Z drain), z0/z1 are separate
tiles and py/yb get private PSUM/SBUF tiles (readers wait ALL writers of
a tile, and pool-slot reuse inherits WAR deps — both serialized the
endgame), and W2Z runs as per-chunk passes chasing the four serial DVE
muls so the PE never dips its p-state. Bias+cast evacs alternate
ACT/DVE; y leaves as four per-co bf16 DMAs on alternating trigger
engines.
"""

import numpy as np
import ml_dtypes

bf16 = ml_dtypes.bfloat16
f8e4 = ml_dtypes.float8_e4m3

# Problem constants (hardcoded per harness contract)
B, C, H, W = 4, 512, 64, 64
D = 512
S = H * W          # 4096 tokens per batch
NCORES = 8
SQ = S * B // NCORES  # 2048 query tokens per core
P = 128            # partitions
NC_C = C // P      # 4 c-chunks
NT = S // P        # 32 t-chunks (keys)
NPAIR = NT // 2    # 16 t-chunk pairs (DoubleRow contracts 2 chunks/pass)
NSQ = SQ // 512    # 4 query tiles of 512
NTT = S // 512     # 8 t-tiles of 512
AHEAD = 4          # sim-ahead-of-Z pipeline depth, in pair units
NWARM = 40         # PE warm-up matmuls (128-col bf16) during the DMA
                   # head; drains ~11.9us, sized to the MEDIAN head
                   # part-A arrival (10.3-12.5us observed): draining
                   # short of a late arrival costs a ramp reset
                   # (~2us of mid-clock U), overshooting an early one
                   # costs only the overshoot
USCALE = 64.0      # fp8 pre-scale for U (undone inside the exp)
HSCALE = 512.0     # fp8 pre-scale for ht
WSCALE = 32.0      # fp8 pre-scale for w2t
LONES = 0.125      # l-ones value: pl = 0.125*l, z = po/pl = 8*(Z/l)
YSCALE = 1.0 / (8.0 * WSCALE)   # y = py*YSCALE + bo'


def build_bass():
    """Build the single-core SPMD Bass program."""
    import concourse.mybir as mybir
    import concourse.tile as tile
    from concourse import bacc

    fp32 = mybir.dt.float32
    bfl = mybir.dt.bfloat16
    f8 = mybir.dt.float8e4
    u8 = mybir.dt.uint8
    AF = mybir.ActivationFunctionType
    DR = mybir.MatmulPerfMode.DoubleRow
    MUL = mybir.AluOpType.mult
    ADD = mybir.AluOpType.add

    nc = bacc.Bacc("TRN2", target_bir_lowering=False)

    # inputs ship pre-tiled: partition dim first, >=2KB runs per partition
    # (wb is 32B/partition but tiny and fires first).
    # head: [P, part, kind, chunk2, 512] u8; part j = contraction chunks
    # 2j:2j+2 of (kind 0 = ht, kind 1 = x tile 0 = query tile 0).
    wb_d = nc.dram_tensor("wb", (P, 8), fp32, kind="ExternalInput")
    head_d = nc.dram_tensor("head", (P, 2, 2, 2, 512), u8, kind="ExternalInput")
    x_d = nc.dram_tensor("x", (P, NTT - 1, NC_C, 512), f8, kind="ExternalInput")
    xt_d = nc.dram_tensor("xt", (P, NT, C), f8, kind="ExternalInput")
    w2t_d = nc.dram_tensor("w2t", (P, NC_C, C), f8, kind="ExternalInput")
    y_d = nc.dram_tensor("y", (NSQ, P, NC_C, 512), bfl, kind="ExternalOutput")

    with tile.TileContext(nc) as tc:
        with (
            tc.tile_pool(name="const", bufs=1) as const,
            tc.tile_pool(name="pt", bufs=9) as ptp,
            tc.tile_pool(name="zsb", bufs=2) as zsb,
            tc.tile_pool(name="padd", bufs=7) as padp,
            tc.tile_pool(name="ybig", bufs=2) as ybp,
            tc.tile_pool(name="small", bufs=8) as small,
            tc.tile_pool(name="ps", bufs=3, space="PSUM") as ps,
            tc.tile_pool(name="psoA", bufs=1, space="PSUM") as psoA,
            tc.tile_pool(name="psoB", bufs=1, space="PSUM") as psoB,
            tc.tile_pool(name="psl", bufs=1, space="PSUM") as psl,
        ):
            # ---- PE warm-up: short 128-col passes, memset on gpsimd (the
            # earliest-starting queue). Keeps the p-state ramp alive until
            # the head lands; count tuned so the queue drains ~then.
            wtile = const.tile([P, P], bfl)
            nc.gpsimd.memset(wtile, 0.01)
            wps = ps.tile([P, 512], fp32, tag="ps")
            for i in range(NWARM):
                nc.tensor.matmul(wps[:, 0:P], wtile, wtile,
                                 start=(i == 0), stop=(i == NWARM - 1))

            # ---- loads: head part A first and 4-way split (it gates U's
            # j=0 passes; more queue slots = bigger bandwidth share), part B
            # next, wb (needed only at the first evac) later on sync.
            head_sb = const.tile([P, 2, 2, 2, 512], u8)
            nc.sync.dma_start(head_sb[0:32, 0], head_d[0:32, 0])
            nc.scalar.dma_start(head_sb[32:64, 0], head_d[32:64, 0])
            nc.sync.dma_start(head_sb[64:96, 0], head_d[64:96, 0])
            nc.scalar.dma_start(head_sb[96:128, 0], head_d[96:128, 0])
            nc.sync.dma_start(head_sb[0:64, 1], head_d[0:64, 1])
            nc.scalar.dma_start(head_sb[64:128, 1], head_d[64:128, 1])
            wb_sb = const.tile([P, 8], fp32)
            nc.sync.dma_start(wb_sb, wb_d[:])

            def wbias(col):  # fp32 scalar-per-partition [P, 1]
                return wb_sb[:, col:col + 1]

            x_sb = const.tile([P, NTT - 1, NC_C, 512], f8)
            xt_sb = const.tile([P, NT, C], f8)
            w2t_sb = const.tile([P, NC_C, C], f8)

            # x tile tt as matmul lhsT operand [P, 2, 128] for (j, col c):
            # tile 0 lives in head (split by j), tiles 1-7 in x_sb.
            def x_lhsT(tt, j, c):
                if tt == 0:
                    return head_sb[:, j, 1].bitcast(f8)[:, :, c * P:(c + 1) * P]
                return x_sb[:, tt - 1, 2 * j:2 * j + 2, c * P:(c + 1) * P]

            # U(st>=1) moving operand [P, 2, 512]: query tile st = x tile st
            def x_mov(st, j):
                return x_sb[:, st - 1, 2 * j:2 * j + 2, :]

            # xt pair 0 + x tile 1 early on scalar (it frees up before the
            # first U evac); bulk on sync, x tiles singly so the first sim
            # tiles never queue behind big descriptors.
            nc.scalar.dma_start(xt_sb[:, 0:2], xt_d[:, 0:2])
            nc.scalar.dma_start(x_sb[:, 0], x_d[:, 0])    # tile 1
            nc.sync.dma_start(x_sb[:, 1], x_d[:, 1])      # tile 2
            nc.sync.dma_start(xt_sb[:, 2:8], xt_d[:, 2:8])
            nc.sync.dma_start(x_sb[:, 2], x_d[:, 2])
            nc.sync.dma_start(xt_sb[:, 8:16], xt_d[:, 8:16])
            nc.sync.dma_start(x_sb[:, 3], x_d[:, 3])
            nc.sync.dma_start(x_sb[:, 4], x_d[:, 4])
            nc.sync.dma_start(xt_sb[:, 16:24], xt_d[:, 16:24])
            nc.sync.dma_start(x_sb[:, 5], x_d[:, 5])
            nc.sync.dma_start(x_sb[:, 6], x_d[:, 6])
            nc.sync.dma_start(xt_sb[:, 24:32], xt_d[:, 24:32])
            nc.sync.dma_start(w2t_sb, w2t_d[:])

            # l-ones: broadcast weights [P, 2, 128] -> pl in all partitions
            # (same PE cost: the pass is bound by the 512-wide moving stream)
            ones8 = const.tile([P, 2, 128], f8)
            nc.vector.memset(ones8, LONES)

            u_sb = const.tile([P, NC_C, SQ], f8)   # U[c, sq] * USCALE, e4m3

            def u_evac(pu, st, co, all_dve=False):
                dst = u_sb[:, co, st * 512:(st + 1) * 512]
                # pu holds HSCALE*(H xq); wt is shipped * USCALE
                if co % 2 == 0 and not all_dve:
                    nc.scalar.activation(dst, pu, AF.Identity,
                                         bias=wbias(co), scale=USCALE / HSCALE)
                else:
                    nc.vector.tensor_scalar(dst, pu, USCALE / HSCALE,
                                            wbias(co), MUL, ADD)

            # U(0): j-outer with 4 live accumulators so the j=0 passes run
            # as soon as head part A lands (psl's bank is free this early)
            def u_proj0():
                pus = [ps.tile([P, 512], fp32, tag="ps", name=f"pu{k}")
                       for k in range(3)]
                pus.append(psl.tile([P, 512], fp32, tag="pl", name="pu3"))
                for j in range(2):
                    ht_j = head_sb[:, j, 0].bitcast(f8)   # [P, 2, 512]
                    mov = head_sb[:, j, 1].bitcast(f8)    # [P, 2, 512]
                    for co in range(NC_C):
                        nc.tensor.matmul(
                            pus[co], ht_j[:, :, co * P:(co + 1) * P], mov,
                            start=(j == 0), stop=(j == 1), perf_mode=DR,
                        )
                        if j == 1:
                            u_evac(pus[co], 0, co)
                    # filler passes on the already-stopped warm tile (a new
                    # group, no pool alloc): after j0 they run while head
                    # part B lands; after j1 they fill the ~0.4us evac
                    # latency before sim(0,0) can start, keeping the ramp
                    # alive in both windows
                    for w in range(2 + j):
                        nc.tensor.matmul(wps[:, 0:P], wtile, wtile,
                                         start=(w == 0), stop=(w == 1 + j))

            def u_proj_co(st, co):
                pu = ps.tile([P, 512], fp32, tag="ps")
                for j in range(2):
                    nc.tensor.matmul(
                        pu,
                        head_sb[:, j, 0].bitcast(f8)[:, :, co * P:(co + 1) * P],
                        x_mov(st, j),
                        start=(j == 0), stop=(j == 1),
                        perf_mode=DR,
                    )
                # mid-kernel boundaries: keep ACT exp-only (same rationale
                # as the y evacs); the head U(0) keeps the ACT/DVE split
                u_evac(pu, st, co, all_dve=True)

            # ---- attention: flat software pipeline over (st, tp) units
            units = [(st, tp) for st in range(NSQ) for tp in range(NPAIR)]
            total = len(units)
            pts = [None] * total
            a1s = {}      # group -> sum of pairs (4g, 4g+1)
            padds = {}    # group -> sum of all 4 pairs
            roots = {}    # st -> sum of the st's first 8 pairs
            state = {}

            def sim_step(i):
                st, tp = units[i]
                pt2 = ptp.tile([P, 2, 512], f8, tag="pt")
                for k in range(2):
                    tch = 2 * tp + k
                    pss = ps.tile([P, 512], fp32, tag="ps")
                    for j in range(2):
                        nc.tensor.matmul(
                            pss,
                            x_lhsT(tch // 4, j, tch % 4),
                            u_sb[:, 2 * j:2 * j + 2, st * 512:(st + 1) * 512],
                            start=(j == 0), stop=(j == 1),
                            perf_mode=DR,
                        )
                    nc.scalar.activation(pt2[:, k, :], pss, AF.Exp,
                                         scale=1.0 / USCALE)
                pts[i] = pt2
                # 4-pair tree-sum for l: add1/add2 on the (otherwise idle)
                # gpsimd engine, add3 on DVE; one l matmul then covers 1024
                # tokens, quartering the PE's l-pass count. Two f8 rounding
                # stages add ~0.1% noise to l (emulated rel err 9.51e-3).
                # The program's very last group skips the tree (its adds
                # would land after the final exps and gate the epilogue's
                # l->recip chain); it uses add1 + two direct passes.
                G = i // 4
                if tp % 4 == 1:
                    pa = padp.tile([P, 2, 512], f8, tag="pa", name="a1")
                    nc.gpsimd.tensor_add(out=pa, in0=pts[i - 1], in1=pt2)
                    a1s[G] = pa
                elif tp % 4 == 3 and i != total - 1:
                    pa2 = padp.tile([P, 2, 512], f8, tag="pa", name="a2")
                    nc.gpsimd.tensor_add(out=pa2, in0=pts[i - 1], in1=pt2)
                    pa3 = padp.tile([P, 2, 512], f8, tag="pa", name="a3")
                    # each st's LAST group keeps add3 on gpsimd: on DVE it
                    # queues ahead of the next U evacs and stalls the next
                    # tile's first sim passes. Two measured dead ends here:
                    # add2-on-DVE (+6us: waits the st's last exps in-queue,
                    # head-of-line blocking later DVE ops) and dropping the
                    # root for two early l passes (+29us: the same
                    # head-of-line wait moves ONTO gpsimd and cascades its
                    # whole queue, slipping every later l group).
                    eng3 = nc.gpsimd if tp == NPAIR - 1 else nc.vector
                    eng3.tensor_add(out=pa3, in0=a1s[G], in1=pa2)
                    padds[G] = pa3
                    if tp == 7:
                        # 8-pair root for the st's first half (mid-st, so
                        # the extra DVE add can't collide with boundary
                        # U evacs or epilogue chains): one more l pass drops
                        pa4 = padp.tile([P, 2, 512], f8, tag="pa", name="a4")
                        nc.vector.tensor_add(out=pa4, in0=padds.pop(G - 1),
                                             in1=pa3)
                        padds.pop(G)
                        roots[st] = pa4

            u_proj0()
            for i in range(AHEAD):
                sim_step(i)
            for i, (st, tp) in enumerate(units):
                na = i + AHEAD
                if na < total:
                    # (spreading these co-groups over the prior st measured
                    # WORSE: the evacs then collide with the l-tree's DVE
                    # adds mid-tile; bunched at the boundary the AHEAD
                    # buffer absorbs the transient ACT spike better)
                    if na % NPAIR == 0:
                        for co in range(NC_C):
                            u_proj_co(na // NPAIR, co)
                    sim_step(na)
                pt2 = pts[i]
                if tp == 0:
                    state["poA"] = psoA.tile([P, 2, 512], fp32, tag="poA", name="poA")
                    state["poB"] = psoB.tile([P, 2, 512], fp32, tag="poB", name="poB")
                    state["pl"] = psl.tile([P, 512], fp32, tag="pl", name="pl")
                poAB, pl = (state["poA"], state["poB"]), state["pl"]
                last = (tp == NPAIR - 1)

                def z_mm(cc):
                    # Z[c, sq] += xt[pair, c-chunk]^T P  (DR, 2 t-chunks);
                    # c-chunks 0,1 -> poA; 2,3 -> poB
                    nc.tensor.matmul(
                        poAB[cc // 2][:, cc % 2, :],
                        xt_sb[:, 2 * tp:2 * tp + 2, cc * P:(cc + 1) * P],
                        pt2,
                        start=(tp == 0), stop=last,
                        perf_mode=DR,
                    )

                def l_mm(p, start, stop):
                    nc.tensor.matmul(pl, ones8, p,
                                     start=start, stop=stop, perf_mode=DR)

                if last and i == total - 1:
                    # final group: add1 (pairs 12-13, ready early) + two
                    # direct passes, emitted first so the reciprocal
                    # overlaps the Z drain
                    l_mm(a1s[i // 4], False, False)
                    l_mm(pts[i - 1], False, False)
                    l_mm(pt2, False, True)
                    for cc in range(NC_C):
                        z_mm(cc)
                elif last:
                    # l first: the reciprocal overlaps the Z drain, and the
                    # next tile's Z passes (gated on the z muls through the
                    # po-pool WAR) start ~1us sooner
                    l_mm(padds.pop(i // 4), False, True)
                    a1s.pop(i // 4, None)
                    for cc in range(NC_C):
                        z_mm(cc)
                else:
                    for cc in range(NC_C):
                        z_mm(cc)
                    if tp == 7:
                        l_mm(roots.pop(st), True, False)
                        a1s.pop(i // 4, None)
                    elif tp == 11:
                        l_mm(padds.pop(i // 4), False, False)
                        a1s.pop(i // 4, None)
                if i != total - 2:   # total-1's direct l pass reads pts[-2]
                    pts[i] = None

                if last:
                    # ---- epilogue for query tile st ----
                    rlb = small.tile([P, 512], fp32, tag="rl")
                    nc.vector.reciprocal_approx_fast(rlb, pl)
                    # z = po * rlb -> e4m3 (DVE only: gpsimd can't read PSUM,
                    # ACT can't tensor*tensor). On the last tile the first
                    # two c-chunks go to SEPARATE tiles so the W2Z chunk-0
                    # passes start after ONE mul (readers wait for all
                    # writers of a tile, so a shared zA tile would gate on
                    # both); zB stays one tile (its muls are last anyway).
                    zlast = (st == NSQ - 1)
                    if zlast:
                        zA = None
                        z0 = small.tile([P, 512], f8, tag="z0")
                        z1 = small.tile([P, 512], f8, tag="z1")
                    else:
                        zA = zsb.tile([P, 2, 512], f8, tag="zA")
                        z0, z1 = zA[:, 0, :], zA[:, 1, :]
                    zB = zsb.tile([P, 2, 512], f8, tag="zB")
                    nc.vector.tensor_mul(out=z0, in0=poAB[0][:, 0, :], in1=rlb)
                    nc.vector.tensor_mul(out=z1, in0=poAB[0][:, 1, :], in1=rlb)
                    nc.vector.tensor_mul(out=zB[:, 0, :], in0=poAB[1][:, 0, :], in1=rlb)
                    nc.vector.tensor_mul(out=zB[:, 1, :], in0=poAB[1][:, 1, :], in1=rlb)

                    # y[c, sq] = (W2 z)*YSCALE + bo'
                    if zlast:
                        # separate PSUM accumulators (ps pool is idle now;
                        # psl's bank frees once the reciprocal reads pl) so
                        # the chunk-0 passes don't inherit the poA/poB pool
                        # slots' WAR dependency on the z muls; separate yb
                        # tiles so the ACT/DVE evacs don't serialize on a
                        # shared tile (writer-after-writer).
                        pycos = [ps.tile([P, 512], fp32, tag="ps",
                                         name=f"py{k}") for k in range(3)]
                        pycos.append(psl.tile([P, 512], fp32, tag="pl",
                                              name="py3"))
                        ybcos = [small.tile([P, 512], bfl, tag="yb",
                                            name=f"yb{k}") for k in range(4)]
                    else:
                        pyA = psoA.tile([P, 2, 512], fp32, tag="poA")
                        pyB = psoB.tile([P, 2, 512], fp32, tag="poB")
                        pys = (pyA, pyB)
                        ybA = ybp.tile([P, 2, 512], bfl, tag="yA")
                        ybB = ybp.tile([P, 2, 512], bfl, tag="yB")
                        ybs = (ybA, ybB)

                    def y_evac(c2):
                        if zlast:
                            dst, src = ybcos[c2], pycos[c2]
                        else:
                            dst = ybs[c2 // 2][:, c2 % 2, :]
                            src = pys[c2 // 2][:, c2 % 2, :]
                        # hidden epilogues: ALL y evacs on DVE — they have
                        # whole-tile slack and queue behind the critical z
                        # muls, while every ACT op at a boundary steals time
                        # from the exp stream (the measured 364ns-pace
                        # bursts). Last tile: split ACT/DVE for parallelism.
                        if zlast and c2 % 2 == 0:
                            nc.scalar.activation(dst, src, AF.Identity,
                                                 bias=wbias(NC_C + c2),
                                                 scale=YSCALE)
                        else:
                            nc.vector.tensor_scalar(dst, src, YSCALE,
                                                    wbias(NC_C + c2), MUL, ADD)

                    if not zlast:
                        # hidden under the next tile's sim: plain DR pairs
                        for j in range(2):
                            for co in range(NC_C):
                                nc.tensor.matmul(
                                    pys[co // 2][:, co % 2, :],
                                    w2t_sb[:, 2 * j:2 * j + 2, co * P:(co + 1) * P],
                                    zA if j == 0 else zB,
                                    start=(j == 0), stop=(j == 1),
                                    perf_mode=DR,
                                )
                                if j == 1 and co % 2 == 1:
                                    y_evac(co - 1)
                                    y_evac(co)
                                    nc.sync.dma_start(
                                        y_d[st, :, co - 1:co + 1], ybs[co // 2])
                    else:
                        # LAST tile: nothing left to hide behind. Per-chunk
                        # passes chase the serial DVE muls (chunk 0 right
                        # after the z0 mul, chunk 1 after z1, then the zB
                        # pair as one DR pass) so the PE never idles long
                        # enough to drop its p-state. Both passes of a
                        # co-pair are emitted BEFORE their evacs: an evac
                        # read of the shared py tile would otherwise gate
                        # the next pair's write (tile-level W-after-R).
                        for zc, chunk in ((z0, 0), (z1, 1)):
                            for co in range(NC_C):
                                nc.tensor.matmul(
                                    pycos[co],
                                    w2t_sb[:, chunk, co * P:(co + 1) * P],
                                    zc,
                                    start=(chunk == 0), stop=False,
                                )
                        for co in range(NC_C):
                            nc.tensor.matmul(
                                pycos[co],
                                w2t_sb[:, 2:4, co * P:(co + 1) * P],
                                zB,
                                start=False, stop=True,
                                perf_mode=DR,
                            )
                            y_evac(co)
                            # per-co DMA, alternating trigger engines: four
                            # queues drain the final 0.5MB in parallel
                            eng = nc.sync if co % 2 == 0 else nc.scalar
                            eng.dma_start(y_d[st, :, co:co + 1], ybcos[co])

    nc.finalize()
    return nc


def make_in_maps(q, Wq, bq, Wk, bk, Wv, bv, Wo, bo):
    """Host-side sharding + weight folding. Returns list of 8 input dicts.

    x ships in a per-core PERMUTED tile order (this core's 4 query tiles
    first), so the kernel's query tile st is always x tile st; xt chunks
    are permuted consistently (any key order is softmax-invariant as long
    as x tiles and xt chunks agree). Tile 0 rides inside `head` with ht.
    """
    scale = float(D) ** -0.5
    x_full = np.ascontiguousarray(q.reshape(B, C, S)).astype(np.float32)

    Hm = Wk.T.astype(np.float32) @ (Wq.astype(np.float32) * scale)   # [c, c]
    wt = Wk.T.astype(np.float32) @ (bq.astype(np.float32) * scale)   # [c]
    W2 = Wo.astype(np.float32) @ Wv.astype(np.float32)               # [c, c]
    bop = Wo.astype(np.float32) @ bv.astype(np.float32) + bo         # [c]

    # [c_in, c_out] -> [p, ci, c_out]
    ht = np.ascontiguousarray(
        (Hm.T * HSCALE).reshape(NC_C, P, C).transpose(1, 0, 2)).astype(f8e4)
    w2t = np.ascontiguousarray(
        (W2.T * WSCALE).reshape(NC_C, P, C).transpose(1, 0, 2)).astype(f8e4)
    wb = np.zeros((P, 8), dtype=np.float32)
    wb[:, 0:NC_C] = (wt * USCALE).reshape(NC_C, P).T
    wb[:, NC_C:2 * NC_C] = bop.reshape(NC_C, P).T

    in_maps = []
    for core in range(NCORES):
        b = core // 2
        h = core % 2
        xb8 = x_full[b].astype(f8e4)                       # [c, s] e4m3
        # x: [c, s] -> [p, tt, o, s512], then permute tiles: queries first
        xh = np.ascontiguousarray(
            xb8.reshape(NC_C, P, NTT, 512).transpose(1, 2, 0, 3))
        perm = list(range(h * 4, h * 4 + 4)) + \
            [t for t in range(NTT) if not h * 4 <= t < h * 4 + 4]
        xp = xh[:, perm]                                   # [p, tt, o, 512]
        # xt: [t, c] -> [p, tch, c], chunks permuted to match x tiles
        xth = np.ascontiguousarray(
            xb8.T.reshape(NT, P, C).transpose(1, 0, 2))
        tchp = [t * 4 + r for t in perm for r in range(4)]
        xtp = np.ascontiguousarray(xth[:, tchp])
        # head[p, part, kind, chunk2, 512]: part j = chunks 2j:2j+2
        head = np.ascontiguousarray(np.stack(
            [np.stack([ht[:, 2 * p:2 * p + 2].view(np.uint8),
                       xp[:, 0, 2 * p:2 * p + 2].view(np.uint8)], axis=1)
             for p in range(2)], axis=1))
        in_maps.append({
            "wb": wb, "head": head,
            "x": np.ascontiguousarray(xp[:, 1:]),
            "xt": xtp, "w2t": w2t,
        })
    return in_maps


def assemble_output(results):
    """results: 8 dicts with 'y' [NSQ, P, NC_C, 512] bf16 -> (B,C,H,W) fp32."""
    y = np.empty((B, C, S), dtype=np.float32)
    for core in range(NCORES):
        b = core // 2
        h = core % 2
        arr = results[core]["y"].astype(np.float32)   # [st, p, co, q]
        y[b][:, h * SQ:(h + 1) * SQ] = (
            arr.transpose(2, 1, 0, 3).reshape(C, SQ))
    return y.reshape(B, C, H, W)


def kernel(**inputs):
    import sys
    for p in ("/opt/trn_rl_repo", "/opt/trn_rl_repo/concourse"):
        if p not in sys.path:
            sys.path.insert(0, p)
    from concourse.bass_utils import run_bass_kernel_spmd

    inputs = {k: np.asarray(v) for k, v in inputs.items()}
    nc = build_bass()
    in_maps = make_in_maps(**inputs)
    res = run_bass_kernel_spmd(nc, in_maps, core_ids=list(range(NCORES)))
    return assemble_output(res.results)


if __name__ == "__main__":
    pass
